# revision 2
# baseline (speedup 1.0000x reference)
"""Trainium2 Bass kernel for nn_AUV_39565238730960.

Computation (per coil c, sharded 1 coil per NeuronCore over 8 cores):
    Z_b   = x_b * csm_c                 (complex elementwise, 30 images)
    Y_b   = T @ Z_b @ T                 (centered ortho 2D FFT as matmuls,
                                         T = symmetric centered DFT matrix)
    Yr    = interleave(Re Y, Im Y)      (30, 131072)
    out_c = mask * (VT^T @ Yr)          (200, 131072)

Implementation notes:
  - FFT matmuls run in float32r (TF32) at 1 cycle/row.
  - Y is stored in SBUF as fp16 (128, 30, 1024); the b->partition corner
    turn is done with per-image SBUF->SBUF DMAs into (30, 16384) tiles.
  - Projection runs in fp16 (lhsT = VT), accumulates fp32 in PSUM.
  - Mask (uint8) is applied by the vector engine during PSUM eviction.
"""

import numpy as np

NCH, NBASIS, NXS, NF = 8, 30, 256, 200
NX = NXS * NXS * 2

_CACHE = {}


def _fmat():
    """Symmetric centered orthonormal DFT matrix: fft1c(z) = T @ z."""
    eye = np.eye(NXS, dtype=np.complex128)
    t = np.fft.fftshift(
        np.fft.fft(np.fft.ifftshift(eye, axes=0), axis=0, norm="ortho"), axes=0
    )
    return t


def _build():
    import concourse.bacc as bacc
    import concourse.mybir as mybir
    import concourse.tile as tile

    F32 = mybir.dt.float32
    F32R = mybir.dt.float32r
    F16 = mybir.dt.float16
    U8 = mybir.dt.uint8
    MULT = mybir.AluOpType.mult
    ADD = mybir.AluOpType.add
    SUB = mybir.AluOpType.subtract

    t = _fmat()
    tr = np.ascontiguousarray(t.real.astype(np.float32).reshape(2, 128, NXS))
    ti = np.ascontiguousarray(t.imag.astype(np.float32).reshape(2, 128, NXS))

    nc = bacc.Bacc("TRN2", target_bir_lowering=False, debug=False, num_devices=NCH)

    x_d = nc.dram_tensor("x", [NBASIS, NXS, NXS, 2], F32, kind="ExternalInput")
    c_d = nc.dram_tensor("csm", [NXS, NXS, 2], F32, kind="ExternalInput")
    v_d = nc.dram_tensor("vt", [NBASIS, NF], F32, kind="ExternalInput")
    m_d = nc.dram_tensor("mask", [NF, NX], U8, kind="ExternalInput")
    o_d = nc.dram_tensor("out", [NF, NX], F32, kind="ExternalOutput")

    tr_d = nc.inline_tensor(tr.transpose(1, 0, 2).copy(), "trmat")  # (128, 2, 256)
    ti_d = nc.inline_tensor(ti.transpose(1, 0, 2).copy(), "timat")
    tn_d = nc.inline_tensor((-ti).transpose(1, 0, 2).copy(), "tnmat")

    NBLK = 4096  # output n-columns per block
    NSUB = 1024  # columns per psum projection tile
    GRP = 4  # blocks per gather group
    NGRP = NX // (NBLK * GRP)  # 8
    NBLOCKS = NX // NBLK  # 32

    with tile.TileContext(nc) as tc:
        with (
            tc.tile_pool(name="const", bufs=1) as cpool,
            tc.tile_pool(name="work", bufs=1) as wpool,
            tc.tile_pool(name="psum", bufs=1, space="PSUM") as psum,
        ):
            # ---- constants ----
            fr = cpool.tile([128, 2, NXS], F32R, name="fr")
            fi = cpool.tile([128, 2, NXS], F32R, name="fi")
            fn = cpool.tile([128, 2, NXS], F32R, name="fn")
            nc.sync.dma_start(fr[:], tr_d.ap().bitcast(F32R))
            nc.sync.dma_start(fi[:], ti_d.ap().bitcast(F32R))
            nc.sync.dma_start(fn[:], tn_d.ap().bitcast(F32R))

            csm = cpool.tile([128, 2, 2 * NXS], F32, name="csm")
            nc.sync.dma_start(
                csm[:], c_d.ap().rearrange("(q p) j r -> p q (j r)", p=128)
            )
            cr = csm[:, :, 0::2]
            ci = csm[:, :, 1::2]

            vt32 = cpool.tile([NBASIS, NF], F32, name="vt32")
            nc.sync.dma_start(vt32[:], v_d.ap())
            vt16 = cpool.tile([NBASIS, NF], F16, name="vt16")
            nc.vector.tensor_copy(vt16[:], vt32[:])

            # Y_all[p, b, kt*512 + col*2 + reim] = Y_b[kt*128+p, col].(re|im)
            y_all = cpool.tile([128, NBASIS, 1024], F16, name="y_all")

            # ---- phase 1: per-image csm-multiply + 2D FFT ----
            for b in range(NBASIS):
                xb = wpool.tile([128, 2, 2 * NXS], F32, name=f"xb{b}", tag="xb", bufs=2)
                nc.sync.dma_start(
                    xb[:], x_d.ap()[b].rearrange("(q p) j r -> p q (j r)", p=128)
                )
                xr = xb[:, :, 0::2]
                xi = xb[:, :, 1::2]

                ta = wpool.tile([128, 2, NXS], F32, name=f"ta{b}", tag="ta", bufs=2)
                tb = wpool.tile([128, 2, NXS], F32, name=f"tb{b}", tag="tb", bufs=2)
                nc.vector.tensor_tensor(ta[:], xr, cr, op=MULT)
                nc.vector.tensor_tensor(tb[:], xi, ci, op=MULT)
                zr = wpool.tile([128, 2, NXS], F32R, name=f"zr{b}", tag="zr", bufs=2)
                nc.vector.tensor_tensor(zr[:], ta[:], tb[:], op=SUB)
                tc_ = wpool.tile([128, 2, NXS], F32, name=f"tc{b}", tag="tc", bufs=2)
                td = wpool.tile([128, 2, NXS], F32, name=f"td{b}", tag="td", bufs=2)
                nc.vector.tensor_tensor(tc_[:], xr, ci, op=MULT)
                nc.vector.tensor_tensor(td[:], xi, cr, op=MULT)
                zi = wpool.tile([128, 2, NXS], F32R, name=f"zi{b}", tag="zi", bufs=2)
                nc.vector.tensor_tensor(zi[:], tc_[:], td[:], op=ADD)

                # pass L: WT[j, k] = sum_i Z[i, j] T[i, k]   (W = T @ Z)
                # wt[p, qj, 0:256] = Re WT[qj*128+p, :], [256:512] = Im WT
                wt = wpool.tile([128, 2, 512], F32R, name=f"wt{b}", tag="wt", bufs=2)
                for jt in range(2):
                    js = slice(jt * 128, (jt + 1) * 128)
                    pl = psum.tile([128, 512], F32, name=f"pl{b}_{jt}", tag="pl", bufs=2)
                    nc.tensor.matmul(pl[:, 0:256], zr[:, 0, js], fr[:, 0, :], start=True, stop=False)
                    nc.tensor.matmul(pl[:, 0:256], zr[:, 1, js], fr[:, 1, :], start=False, stop=False)
                    nc.tensor.matmul(pl[:, 0:256], zi[:, 0, js], fn[:, 0, :], start=False, stop=False)
                    nc.tensor.matmul(pl[:, 0:256], zi[:, 1, js], fn[:, 1, :], start=False, stop=True)
                    nc.tensor.matmul(pl[:, 256:512], zr[:, 0, js], fi[:, 0, :], start=True, stop=False)
                    nc.tensor.matmul(pl[:, 256:512], zr[:, 1, js], fi[:, 1, :], start=False, stop=False)
                    nc.tensor.matmul(pl[:, 256:512], zi[:, 0, js], fr[:, 0, :], start=False, stop=False)
                    nc.tensor.matmul(pl[:, 256:512], zi[:, 1, js], fr[:, 1, :], start=False, stop=True)
                    nc.scalar.copy(wt[:, jt, :], pl[:])

                # pass R: Y[k, n] = sum_j WT[j, k] T[j, n]   (Y = W @ T)
                for kt in range(2):
                    ksr = slice(kt * 128, (kt + 1) * 128)
                    ksi = slice(256 + kt * 128, 256 + (kt + 1) * 128)
                    pr = psum.tile([128, 512], F32, name=f"pr{b}_{kt}", tag="pr", bufs=2)
                    nc.tensor.matmul(pr[:, 0:256], wt[:, 0, ksr], fr[:, 0, :], start=True, stop=False)
                    nc.tensor.matmul(pr[:, 0:256], wt[:, 1, ksr], fr[:, 1, :], start=False, stop=False)
                    nc.tensor.matmul(pr[:, 0:256], wt[:, 0, ksi], fn[:, 0, :], start=False, stop=False)
                    nc.tensor.matmul(pr[:, 0:256], wt[:, 1, ksi], fn[:, 1, :], start=False, stop=True)
                    nc.tensor.matmul(pr[:, 256:512], wt[:, 0, ksr], fi[:, 0, :], start=True, stop=False)
                    nc.tensor.matmul(pr[:, 256:512], wt[:, 1, ksr], fi[:, 1, :], start=False, stop=False)
                    nc.tensor.matmul(pr[:, 256:512], wt[:, 0, ksi], fr[:, 0, :], start=False, stop=False)
                    nc.tensor.matmul(pr[:, 256:512], wt[:, 1, ksi], fr[:, 1, :], start=False, stop=True)
                    # interleave re/im while evicting: y[.., col*2+ri] = pr[.., ri*256+col]
                    nc.scalar.copy(
                        y_all[:, b, kt * 512 : (kt + 1) * 512].rearrange(
                            "p (c r) -> p c r", r=2
                        ),
                        pr[:].rearrange("p (r c) -> p c r", r=2),
                    )

            # ---- phase 2: corner turn + projection + mask + store ----
            for g in range(NGRP):
                ch = g // 4
                p0 = (g % 4) * 32
                yg = wpool.tile(
                    [NBASIS, GRP * NBLK], F16, name=f"yg{g}", tag="yg", bufs=2
                )
                for b in range(NBASIS):
                    nc.scalar.dma_start(
                        yg[b : b + 1, :].rearrange("b (p f) -> b p f", p=32),
                        y_all[p0 : p0 + 32, b, ch * 512 : (ch + 1) * 512],
                    )
                for j in range(GRP):
                    blk = g * GRP + j
                    n0 = blk * NBLK
                    m0 = wpool.tile([128, NBLK], U8, name=f"m0_{blk}", tag="m0", bufs=1)
                    m1 = wpool.tile([128, NBLK], U8, name=f"m1_{blk}", tag="m1", bufs=1)
                    nc.sync.dma_start(m0[:], m_d.ap()[0:128, n0 : n0 + NBLK])
                    nc.sync.dma_start(m1[0:72, :], m_d.ap()[128:200, n0 : n0 + NBLK])
                    for sub in range(NBLK // NSUB):
                        off = j * NBLK + sub * NSUB
                        c0 = n0 + sub * NSUB
                        for ft, nf in ((0, 128), (1, 72)):
                            pp = psum.tile(
                                [128, NSUB],
                                F32,
                                name=f"pp{blk}_{sub}_{ft}",
                                tag="pp",
                                bufs=2,
                            )
                            fsl = slice(ft * 128, ft * 128 + nf)
                            nc.tensor.matmul(
                                pp[0:nf, 0:512],
                                vt16[:, fsl],
                                yg[0:NBASIS, off : off + 512],
                                start=True,
                                stop=True,
                            )
                            nc.tensor.matmul(
                                pp[0:nf, 512:1024],
                                vt16[:, fsl],
                                yg[0:NBASIS, off + 512 : off + 1024],
                                start=True,
                                stop=True,
                            )
                            msk = (m0 if ft == 0 else m1)[
                                0:nf, sub * NSUB : (sub + 1) * NSUB
                            ]
                            ob = wpool.tile(
                                [128, NSUB],
                                F32,
                                name=f"ob{blk}_{sub}_{ft}",
                                tag=f"ob{ft}",
                                bufs=2,
                            )
                            nc.vector.tensor_tensor(
                                ob[0:nf, :], pp[0:nf, :], msk, op=MULT
                            )
                            nc.sync.dma_start(
                                o_d.ap()[
                                    ft * 128 : ft * 128 + nf, c0 : c0 + NSUB
                                ],
                                ob[0:nf, :],
                            )

    nc.compile()
    return nc


def _get_nc():
    if "nc" not in _CACHE:
        _CACHE["nc"] = _build()
    return _CACHE["nc"]


def kernel(x, csmT, VT, maskT):
    from concourse import bass2jax

    x = np.ascontiguousarray(np.asarray(x, dtype=np.float32))
    csm = np.ascontiguousarray(np.asarray(csmT, dtype=np.float32))
    vt = np.ascontiguousarray(np.asarray(VT, dtype=np.float32))
    mask = np.ascontiguousarray(np.asarray(maskT)).view(np.uint8)

    nc = _get_nc()
    in_maps = [
        {"x": x, "csm": csm[c], "vt": vt, "mask": mask} for c in range(NCH)
    ]
    results = bass2jax.run_bass_via_pjrt(nc, in_maps, n_cores=NCH)
    return np.stack([results[c]["out"] for c in range(NCH)], axis=0)


# revision 6
# speedup vs baseline: 1.2988x; 1.2988x over previous
"""Trainium2 Bass kernel for nn_AUV_39565238730960.

Computation (per coil c, sharded 1 coil per NeuronCore over 8 cores):
    Z_b   = x_b * csm_c                 (complex elementwise, 30 images)
    Y_b   = T @ Z_b @ T                 (centered ortho 2D FFT as matmuls,
                                         T = symmetric centered DFT matrix)
    Yr    = interleave(Re Y, Im Y)      (30, 131072)
    out_c = mask * (VT^T @ Yr)          (200, 131072)

Implementation notes:
  - FFT pass L runs in float32r (TF32), pass R in fp16; both use stacked
    moving operands [Tr|Ti] / [-Ti|Tr] so each complex matmul pair is a
    single N=512 instruction accumulating into one PSUM bank.
  - The b->partition corner turn bounces through a DRAM scratch tensor
    (30, 131072) fp16: one scatter DMA per image, one block read per
    output block group.
  - Projection runs in fp16 (lhsT = VT), accumulates fp32 in PSUM.
  - Mask (uint8) is applied by the vector engine during PSUM eviction.
"""

import numpy as np

NCH, NBASIS, NXS, NF = 8, 30, 256, 200
NX = NXS * NXS * 2

_CACHE = {}


def _fmat():
    """Symmetric centered orthonormal DFT matrix: fft1c(z) = T @ z."""
    eye = np.eye(NXS, dtype=np.complex128)
    t = np.fft.fftshift(
        np.fft.fft(np.fft.ifftshift(eye, axes=0), axis=0, norm="ortho"), axes=0
    )
    return t


def _build():
    import concourse.bacc as bacc
    import concourse.mybir as mybir
    import concourse.tile as tile

    F32 = mybir.dt.float32
    F32R = mybir.dt.float32r
    F16 = mybir.dt.float16
    U8 = mybir.dt.uint8
    MULT = mybir.AluOpType.mult
    ADD = mybir.AluOpType.add
    SUB = mybir.AluOpType.subtract

    t = _fmat()
    tr = t.real.astype(np.float32).reshape(2, 128, NXS).transpose(1, 0, 2)
    ti = t.imag.astype(np.float32).reshape(2, 128, NXS).transpose(1, 0, 2)
    # stacked moving operands (128, 2, 512): [Tr | Ti] and [-Ti | Tr]
    f_a = np.concatenate([tr, ti], axis=2).copy()
    f_b = np.concatenate([-ti, tr], axis=2).copy()

    nc = bacc.Bacc("TRN2", target_bir_lowering=False, debug=False, num_devices=NCH)

    x_d = nc.dram_tensor("x", [NBASIS, NXS, NXS, 2], F32, kind="ExternalInput")
    c_d = nc.dram_tensor("csm", [NXS, NXS, 2], F32, kind="ExternalInput")
    v_d = nc.dram_tensor("vt", [NBASIS, NF], F32, kind="ExternalInput")
    m_d = nc.dram_tensor("mask", [NF, NX], U8, kind="ExternalInput")
    o_d = nc.dram_tensor("out", [NF, NX], F32, kind="ExternalOutput")

    fa_d = nc.inline_tensor(f_a, "fmat_a")
    fb_d = nc.inline_tensor(f_b, "fmat_b")
    fa16_d = nc.inline_tensor(f_a.astype(np.float16), "fmat_a16")
    fb16_d = nc.inline_tensor(f_b.astype(np.float16), "fmat_b16")

    # DRAM scratch for the b->partition corner turn, fp16 row-major (b, n)
    s_d = nc.dram_tensor("yscratch", [NBASIS, NX], F16)

    NBLK = 4096  # mask/out tile width
    NGATH = 8192  # yr block-read width
    NSUB = 1024  # psum projection tile width

    with tile.TileContext(nc) as tc:
        with (
            tc.tile_pool(name="const", bufs=1) as cpool,
            tc.tile_pool(name="work", bufs=1) as wpool,
            tc.tile_pool(name="psum", bufs=1, space="PSUM") as psum,
        ):
            # ---- constants ----
            fa = cpool.tile([128, 2, 512], F32R, name="fa")
            fb = cpool.tile([128, 2, 512], F32R, name="fb")
            fa16 = cpool.tile([128, 2, 512], F16, name="fa16")
            fb16 = cpool.tile([128, 2, 512], F16, name="fb16")
            nc.sync.dma_start(fa[:], fa_d.ap().bitcast(F32R))
            nc.sync.dma_start(fb[:], fb_d.ap().bitcast(F32R))
            nc.sync.dma_start(fa16[:], fa16_d.ap())
            nc.sync.dma_start(fb16[:], fb16_d.ap())

            csm = cpool.tile([128, 2, 2 * NXS], F32, name="csm")
            nc.sync.dma_start(
                csm[:], c_d.ap().rearrange("(q p) j r -> p q (j r)", p=128)
            )
            cr = csm[:, :, 0::2]
            ci = csm[:, :, 1::2]

            vt32 = cpool.tile([NBASIS, NF], F32, name="vt32")
            nc.sync.dma_start(vt32[:], v_d.ap())
            vt16 = cpool.tile([NBASIS, NF], F16, name="vt16")
            nc.vector.tensor_copy(vt16[:], vt32[:])

            # ---- phase 1: per-image csm-multiply + 2D FFT -> DRAM scratch ----
            for b in range(NBASIS):
                xb = wpool.tile([128, 2, 2 * NXS], F32, name=f"xb{b}", tag="xb", bufs=3)
                nc.scalar.dma_start(
                    xb[:], x_d.ap()[b].rearrange("(q p) j r -> p q (j r)", p=128)
                )
                xr = xb[:, :, 0::2]
                xi = xb[:, :, 1::2]

                ta = wpool.tile([128, 2, NXS], F32, name=f"ta{b}", tag="ta", bufs=2)
                tb = wpool.tile([128, 2, NXS], F32, name=f"tb{b}", tag="tb", bufs=2)
                nc.vector.tensor_tensor(ta[:], xr, cr, op=MULT)
                nc.vector.tensor_tensor(tb[:], xi, ci, op=MULT)
                zr = wpool.tile([128, 2, NXS], F32R, name=f"zr{b}", tag="zr", bufs=3)
                nc.vector.tensor_tensor(zr[:], ta[:], tb[:], op=SUB)
                tc_ = wpool.tile([128, 2, NXS], F32, name=f"tc{b}", tag="tc", bufs=2)
                td = wpool.tile([128, 2, NXS], F32, name=f"td{b}", tag="td", bufs=2)
                nc.vector.tensor_tensor(tc_[:], xr, ci, op=MULT)
                nc.vector.tensor_tensor(td[:], xi, cr, op=MULT)
                zi = wpool.tile([128, 2, NXS], F32R, name=f"zi{b}", tag="zi", bufs=3)
                nc.vector.tensor_tensor(zi[:], tc_[:], td[:], op=ADD)

                # pass L: WT[j, k] = sum_i Z[i, j] T[i, k]   (W = T @ Z)
                # wt[p, qj, 0:256] = Re WT[qj*128+p, :], [256:512] = Im WT
                wt = wpool.tile([128, 2, 512], F16, name=f"wt{b}", tag="wt", bufs=2)
                for jt in range(2):
                    js = slice(jt * 128, (jt + 1) * 128)
                    pl = psum.tile([128, 512], F32, name=f"pl{b}_{jt}", tag="pl", bufs=2)
                    for half, m1, m2 in ((slice(0, 256), fa, fb), (slice(256, 512), fa, fb)):
                        h0 = 0 if half.start == 0 else 256
                        nc.tensor.matmul(pl[:, half], zr[:, 0, js], m1[:, 0, h0 : h0 + 256], start=True, stop=False)
                        nc.tensor.matmul(pl[:, half], zr[:, 1, js], m1[:, 1, h0 : h0 + 256], start=False, stop=False)
                        nc.tensor.matmul(pl[:, half], zi[:, 0, js], m2[:, 0, h0 : h0 + 256], start=False, stop=False)
                        nc.tensor.matmul(pl[:, half], zi[:, 1, js], m2[:, 1, h0 : h0 + 256], start=False, stop=True)
                    nc.scalar.copy(wt[:, jt, :], pl[:])

                # pass R: Y[k, n] = sum_j WT[j, k] T[j, n]   (Y = W @ T)
                yb = wpool.tile([128, 2, 512], F16, name=f"yb{b}", tag="yb", bufs=2)
                for kt in range(2):
                    ksr = slice(kt * 128, (kt + 1) * 128)
                    ksi = slice(256 + kt * 128, 256 + (kt + 1) * 128)
                    pr = psum.tile([128, 512], F32, name=f"pr{b}_{kt}", tag="pr", bufs=2)
                    nc.tensor.matmul(pr[:], wt[:, 0, ksr], fa16[:, 0, :], start=True, stop=False)
                    nc.tensor.matmul(pr[:], wt[:, 1, ksr], fa16[:, 1, :], start=False, stop=False)
                    nc.tensor.matmul(pr[:], wt[:, 0, ksi], fb16[:, 0, :], start=False, stop=False)
                    nc.tensor.matmul(pr[:], wt[:, 1, ksi], fb16[:, 1, :], start=False, stop=True)
                    # interleave re/im while evicting: y[.., col*2+ri] = pr[.., ri*256+col]
                    nc.scalar.copy(
                        yb[:, kt, :].rearrange("p (c r) -> p c r", r=2),
                        pr[:].rearrange("p (r c) -> p c r", r=2),
                    )
                # scatter to scratch: scratch[b, (kt*128+p)*512 + c] = yb[p, kt, c]
                nc.scalar.dma_start(
                    s_d.ap()[b].rearrange("(kt p c) -> p kt c", p=128, kt=2),
                    yb[:],
                )

            # ---- phase 2: block read + projection + mask + store ----
            for g in range(NX // NGATH):  # 16 groups
                yg = wpool.tile([NBASIS, NGATH], F16, name=f"yg{g}", tag="yg", bufs=2)
                nc.scalar.dma_start(
                    yg[:], s_d.ap()[:, g * NGATH : (g + 1) * NGATH]
                )
                for j in range(NGATH // NBLK):  # 2 blocks per group
                    blk = g * 2 + j
                    n0 = blk * NBLK
                    m0 = wpool.tile([128, NBLK], U8, name=f"m0_{blk}", tag="m0", bufs=2)
                    m1 = wpool.tile([128, NBLK], U8, name=f"m1_{blk}", tag="m1", bufs=2)
                    nc.sync.dma_start(m0[:], m_d.ap()[0:128, n0 : n0 + NBLK])
                    nc.sync.dma_start(m1[0:72, :], m_d.ap()[128:200, n0 : n0 + NBLK])
                    ob0 = wpool.tile([128, NBLK], F32, name=f"ob0_{blk}", tag="ob0", bufs=2)
                    ob1 = wpool.tile([128, NBLK], F32, name=f"ob1_{blk}", tag="ob1", bufs=2)
                    for sub in range(NBLK // NSUB):  # 4 psum tiles per block
                        off = j * NBLK + sub * NSUB
                        ssl = slice(sub * NSUB, (sub + 1) * NSUB)
                        for ft, nf, ob, msk in ((0, 128, ob0, m0), (1, 72, ob1, m1)):
                            pp = psum.tile(
                                [128, NSUB],
                                F32,
                                name=f"pp{blk}_{sub}_{ft}",
                                tag="pp",
                                bufs=2,
                            )
                            fsl = slice(ft * 128, ft * 128 + nf)
                            nc.tensor.matmul(
                                pp[0:nf, 0:512],
                                vt16[:, fsl],
                                yg[0:NBASIS, off : off + 512],
                                start=True,
                                stop=True,
                            )
                            nc.tensor.matmul(
                                pp[0:nf, 512:1024],
                                vt16[:, fsl],
                                yg[0:NBASIS, off + 512 : off + 1024],
                                start=True,
                                stop=True,
                            )
                            nc.vector.tensor_tensor(
                                ob[0:nf, ssl], pp[0:nf, :], msk[0:nf, ssl], op=MULT
                            )
                    nc.sync.dma_start(o_d.ap()[0:128, n0 : n0 + NBLK], ob0[:])
                    nc.sync.dma_start(o_d.ap()[128:200, n0 : n0 + NBLK], ob1[0:72, :])

    nc.compile()
    return nc


def _get_nc():
    if "nc" not in _CACHE:
        _CACHE["nc"] = _build()
    return _CACHE["nc"]


def kernel(x, csmT, VT, maskT):
    from concourse import bass2jax

    x = np.ascontiguousarray(np.asarray(x, dtype=np.float32))
    csm = np.ascontiguousarray(np.asarray(csmT, dtype=np.float32))
    vt = np.ascontiguousarray(np.asarray(VT, dtype=np.float32))
    mask = np.ascontiguousarray(np.asarray(maskT)).view(np.uint8)

    nc = _get_nc()
    in_maps = [
        {"x": x, "csm": csm[c], "vt": vt, "mask": mask} for c in range(NCH)
    ]
    results = bass2jax.run_bass_via_pjrt(nc, in_maps, n_cores=NCH)
    return np.stack([results[c]["out"] for c in range(NCH)], axis=0)


# revision 7
# speedup vs baseline: 1.5411x; 1.1866x over previous
"""Trainium2 Bass kernel for nn_AUV_39565238730960.

Computation (per coil c, sharded 1 coil per NeuronCore over 8 cores):
    Z_b   = x_b * csm_c                 (complex elementwise, 30 images)
    Y_b   = T @ Z_b @ T                 (centered ortho 2D FFT as matmuls,
                                         T = symmetric centered DFT matrix)
    Yr    = interleave(Re Y, Im Y)      (30, 131072)
    out_c = mask * (VT^T @ Yr)          (200, 131072)

Implementation notes:
  - FFT pass L runs in float32r (TF32), pass R in fp16; both use stacked
    moving operands [Tr|Ti] / [-Ti|Tr] so each complex matmul pair is a
    single N=512 instruction accumulating into one PSUM bank.
  - The b->partition corner turn bounces through a DRAM scratch tensor
    (30, 131072) fp16: one scatter DMA per image, one block read per
    output block group.
  - Projection runs in fp16 (lhsT = VT), accumulates fp32 in PSUM.
  - Mask (uint8) is applied by the vector engine during PSUM eviction.
"""

import numpy as np

NCH, NBASIS, NXS, NF = 8, 30, 256, 200
NX = NXS * NXS * 2

_CACHE = {}


def _fmat():
    """Symmetric centered orthonormal DFT matrix: fft1c(z) = T @ z."""
    eye = np.eye(NXS, dtype=np.complex128)
    t = np.fft.fftshift(
        np.fft.fft(np.fft.ifftshift(eye, axes=0), axis=0, norm="ortho"), axes=0
    )
    return t


def _build():
    import concourse.bacc as bacc
    import concourse.mybir as mybir
    import concourse.tile as tile

    F32 = mybir.dt.float32
    F32R = mybir.dt.float32r
    F16 = mybir.dt.float16
    U8 = mybir.dt.uint8
    MULT = mybir.AluOpType.mult
    ADD = mybir.AluOpType.add
    SUB = mybir.AluOpType.subtract

    t = _fmat()
    tr = t.real.astype(np.float32).reshape(2, 128, NXS).transpose(1, 0, 2)
    ti = t.imag.astype(np.float32).reshape(2, 128, NXS).transpose(1, 0, 2)
    # stacked moving operands (128, 2, 512): [Tr | Ti] and [-Ti | Tr]
    f_a = np.concatenate([tr, ti], axis=2).copy()
    f_b = np.concatenate([-ti, tr], axis=2).copy()

    nc = bacc.Bacc("TRN2", target_bir_lowering=False, debug=False, num_devices=NCH)

    x_d = nc.dram_tensor("x", [NBASIS, NXS, NXS, 2], F32, kind="ExternalInput")
    c_d = nc.dram_tensor("csm", [NXS, NXS, 2], F32, kind="ExternalInput")
    v_d = nc.dram_tensor("vt", [NBASIS, NF], F32, kind="ExternalInput")
    m_d = nc.dram_tensor("mask", [NF, NX], U8, kind="ExternalInput")
    o_d = nc.dram_tensor("out", [NF, NX], F32, kind="ExternalOutput")

    fa_d = nc.inline_tensor(f_a, "fmat_a")
    fb_d = nc.inline_tensor(f_b, "fmat_b")
    fa16_d = nc.inline_tensor(f_a.astype(np.float16), "fmat_a16")
    fb16_d = nc.inline_tensor(f_b.astype(np.float16), "fmat_b16")

    # DRAM scratch for the b->partition corner turn, fp16 row-major (b, n)
    s_d = nc.dram_tensor("yscratch", [NBASIS, NX], F16)

    NBLK = 4096  # mask/out tile width
    NGATH = 8192  # yr block-read width
    NSUB = 1024  # psum projection tile width

    with tile.TileContext(nc) as tc:
        with (
            tc.tile_pool(name="const", bufs=1) as cpool,
            tc.tile_pool(name="work", bufs=1) as wpool,
            tc.tile_pool(name="psum", bufs=1, space="PSUM") as psum,
        ):
            # ---- constants ----
            fa16 = cpool.tile([128, 2, 512], F16, name="fa16")
            fb16 = cpool.tile([128, 2, 512], F16, name="fb16")
            nc.sync.dma_start(fa16[:], fa16_d.ap())
            nc.sync.dma_start(fb16[:], fb16_d.ap())

            csm = cpool.tile([128, 2, 2 * NXS], F32, name="csm")
            nc.sync.dma_start(
                csm[:], c_d.ap().rearrange("(q p) j r -> p q (j r)", p=128)
            )
            cr = csm[:, :, 0::2]
            ci = csm[:, :, 1::2]

            vt32 = cpool.tile([NBASIS, NF], F32, name="vt32")
            nc.sync.dma_start(vt32[:], v_d.ap())
            vt16 = cpool.tile([NBASIS, NF], F16, name="vt16")
            nc.vector.tensor_copy(vt16[:], vt32[:])

            # ---- phase 1: per-image csm-multiply + 2D FFT -> DRAM scratch ----
            for b in range(NBASIS):
                xb = wpool.tile([128, 2, 2 * NXS], F32, name=f"xb{b}", tag="xb", bufs=3)
                nc.scalar.dma_start(
                    xb[:], x_d.ap()[b].rearrange("(q p) j r -> p q (j r)", p=128)
                )
                xr = xb[:, :, 0::2]
                xi = xb[:, :, 1::2]

                ta = wpool.tile([128, 2, NXS], F32, name=f"ta{b}", tag="ta", bufs=2)
                tb = wpool.tile([128, 2, NXS], F32, name=f"tb{b}", tag="tb", bufs=2)
                nc.vector.tensor_tensor(ta[:], xr, cr, op=MULT)
                nc.vector.tensor_tensor(tb[:], xi, ci, op=MULT)
                zr = wpool.tile([128, 2, NXS], F16, name=f"zr{b}", tag="zr", bufs=3)
                nc.vector.tensor_tensor(zr[:], ta[:], tb[:], op=SUB)
                tc_ = wpool.tile([128, 2, NXS], F32, name=f"tc{b}", tag="tc", bufs=2)
                td = wpool.tile([128, 2, NXS], F32, name=f"td{b}", tag="td", bufs=2)
                nc.vector.tensor_tensor(tc_[:], xr, ci, op=MULT)
                nc.vector.tensor_tensor(td[:], xi, cr, op=MULT)
                zi = wpool.tile([128, 2, NXS], F16, name=f"zi{b}", tag="zi", bufs=3)
                nc.vector.tensor_tensor(zi[:], tc_[:], td[:], op=ADD)

                # pass L: WT[j, k] = sum_i Z[i, j] T[i, k]   (W = T @ Z)
                # wt[p, qj, 0:256] = Re WT[qj*128+p, :], [256:512] = Im WT
                wt = wpool.tile([128, 2, 512], F16, name=f"wt{b}", tag="wt", bufs=2)
                for jt in range(2):
                    js = slice(jt * 128, (jt + 1) * 128)
                    pl = psum.tile([128, 512], F32, name=f"pl{b}_{jt}", tag="pl", bufs=2)
                    nc.tensor.matmul(pl[:], zr[:, 0, js], fa16[:, 0, :], start=True, stop=False)
                    nc.tensor.matmul(pl[:], zr[:, 1, js], fa16[:, 1, :], start=False, stop=False)
                    nc.tensor.matmul(pl[:], zi[:, 0, js], fb16[:, 0, :], start=False, stop=False)
                    nc.tensor.matmul(pl[:], zi[:, 1, js], fb16[:, 1, :], start=False, stop=True)
                    nc.scalar.copy(wt[:, jt, :], pl[:])

                # pass R: Y[k, n] = sum_j WT[j, k] T[j, n]   (Y = W @ T)
                yb = wpool.tile([128, 2, 512], F16, name=f"yb{b}", tag="yb", bufs=2)
                for kt in range(2):
                    ksr = slice(kt * 128, (kt + 1) * 128)
                    ksi = slice(256 + kt * 128, 256 + (kt + 1) * 128)
                    pr = psum.tile([128, 512], F32, name=f"pr{b}_{kt}", tag="pr", bufs=2)
                    nc.tensor.matmul(pr[:], wt[:, 0, ksr], fa16[:, 0, :], start=True, stop=False)
                    nc.tensor.matmul(pr[:], wt[:, 1, ksr], fa16[:, 1, :], start=False, stop=False)
                    nc.tensor.matmul(pr[:], wt[:, 0, ksi], fb16[:, 0, :], start=False, stop=False)
                    nc.tensor.matmul(pr[:], wt[:, 1, ksi], fb16[:, 1, :], start=False, stop=True)
                    # interleave re/im while evicting: y[.., col*2+ri] = pr[.., ri*256+col]
                    nc.scalar.copy(
                        yb[:, kt, :].rearrange("p (c r) -> p c r", r=2),
                        pr[:].rearrange("p (r c) -> p c r", r=2),
                    )
                # scatter to scratch: scratch[b, (kt*128+p)*512 + c] = yb[p, kt, c]
                nc.scalar.dma_start(
                    s_d.ap()[b].rearrange("(kt p c) -> p kt c", p=128, kt=2),
                    yb[:],
                )

            # ---- phase 2: block read + projection + mask + store ----
            for g in range(NX // NGATH):  # 16 groups
                yg = wpool.tile([NBASIS, NGATH], F16, name=f"yg{g}", tag="yg", bufs=2)
                nc.scalar.dma_start(
                    yg[:], s_d.ap()[:, g * NGATH : (g + 1) * NGATH]
                )
                for j in range(NGATH // NBLK):  # 2 blocks per group
                    blk = g * 2 + j
                    n0 = blk * NBLK
                    m0 = wpool.tile([128, NBLK], U8, name=f"m0_{blk}", tag="m0", bufs=2)
                    m1 = wpool.tile([128, NBLK], U8, name=f"m1_{blk}", tag="m1", bufs=2)
                    nc.scalar.dma_start(m0[:], m_d.ap()[0:128, n0 : n0 + NBLK])
                    nc.scalar.dma_start(m1[0:72, :], m_d.ap()[128:200, n0 : n0 + NBLK])
                    ob0 = wpool.tile([128, NBLK], F32, name=f"ob0_{blk}", tag="ob0", bufs=2)
                    ob1 = wpool.tile([128, NBLK], F32, name=f"ob1_{blk}", tag="ob1", bufs=2)
                    for sub in range(NBLK // NSUB):  # 4 psum tiles per block
                        off = j * NBLK + sub * NSUB
                        ssl = slice(sub * NSUB, (sub + 1) * NSUB)
                        for ft, nf, ob, msk in ((0, 128, ob0, m0), (1, 72, ob1, m1)):
                            pp = psum.tile(
                                [128, NSUB],
                                F32,
                                name=f"pp{blk}_{sub}_{ft}",
                                tag="pp",
                                bufs=2,
                            )
                            fsl = slice(ft * 128, ft * 128 + nf)
                            nc.tensor.matmul(
                                pp[0:nf, 0:512],
                                vt16[:, fsl],
                                yg[0:NBASIS, off : off + 512],
                                start=True,
                                stop=True,
                            )
                            nc.tensor.matmul(
                                pp[0:nf, 512:1024],
                                vt16[:, fsl],
                                yg[0:NBASIS, off + 512 : off + 1024],
                                start=True,
                                stop=True,
                            )
                            nc.vector.tensor_tensor(
                                ob[0:nf, ssl], pp[0:nf, :], msk[0:nf, ssl], op=MULT
                            )
                    nc.sync.dma_start(o_d.ap()[0:128, n0 : n0 + NBLK], ob0[:])
                    nc.sync.dma_start(o_d.ap()[128:200, n0 : n0 + NBLK], ob1[0:72, :])

    nc.compile()
    return nc


def _get_nc():
    if "nc" not in _CACHE:
        _CACHE["nc"] = _build()
    return _CACHE["nc"]


def kernel(x, csmT, VT, maskT):
    from concourse import bass2jax

    x = np.ascontiguousarray(np.asarray(x, dtype=np.float32))
    csm = np.ascontiguousarray(np.asarray(csmT, dtype=np.float32))
    vt = np.ascontiguousarray(np.asarray(VT, dtype=np.float32))
    mask = np.ascontiguousarray(np.asarray(maskT)).view(np.uint8)

    nc = _get_nc()
    in_maps = [
        {"x": x, "csm": csm[c], "vt": vt, "mask": mask} for c in range(NCH)
    ]
    results = bass2jax.run_bass_via_pjrt(nc, in_maps, n_cores=NCH)
    return np.stack([results[c]["out"] for c in range(NCH)], axis=0)


# revision 9
# speedup vs baseline: 1.6197x; 1.0510x over previous
"""Trainium2 Bass kernel for nn_AUV_39565238730960.

Computation (per coil c, sharded 1 coil per NeuronCore over 8 cores):
    Z_b   = x_b * csm_c                 (complex elementwise, 30 images)
    Y_b   = T @ Z_b @ T                 (centered ortho 2D FFT as matmuls,
                                         T = symmetric centered DFT matrix)
    Yr    = interleave(Re Y, Im Y)      (30, 131072)
    out_c = mask * (VT^T @ Yr)          (200, 131072)

Implementation notes:
  - FFT pass L runs in float32r (TF32), pass R in fp16; both use stacked
    moving operands [Tr|Ti] / [-Ti|Tr] so each complex matmul pair is a
    single N=512 instruction accumulating into one PSUM bank.
  - The b->partition corner turn bounces through a DRAM scratch tensor
    (30, 131072) fp16: one scatter DMA per image, one block read per
    output block group.
  - Projection runs in fp16 (lhsT = VT), accumulates fp32 in PSUM.
  - Mask (uint8) is applied by the vector engine during PSUM eviction.
"""

import numpy as np

NCH, NBASIS, NXS, NF = 8, 30, 256, 200
NX = NXS * NXS * 2

_CACHE = {}


def _fmat():
    """Symmetric centered orthonormal DFT matrix: fft1c(z) = T @ z."""
    eye = np.eye(NXS, dtype=np.complex128)
    t = np.fft.fftshift(
        np.fft.fft(np.fft.ifftshift(eye, axes=0), axis=0, norm="ortho"), axes=0
    )
    return t


def _build():
    import concourse.bacc as bacc
    import concourse.mybir as mybir
    import concourse.tile as tile

    F32 = mybir.dt.float32
    F32R = mybir.dt.float32r
    F16 = mybir.dt.float16
    U8 = mybir.dt.uint8
    MULT = mybir.AluOpType.mult
    ADD = mybir.AluOpType.add
    SUB = mybir.AluOpType.subtract

    t = _fmat()
    tr = t.real.astype(np.float32).reshape(2, 128, NXS).transpose(1, 0, 2)
    ti = t.imag.astype(np.float32).reshape(2, 128, NXS).transpose(1, 0, 2)
    # stacked moving operands (128, 2, 512): [Tr | Ti] and [-Ti | Tr]
    f_a = np.concatenate([tr, ti], axis=2).copy()
    f_b = np.concatenate([-ti, tr], axis=2).copy()

    nc = bacc.Bacc("TRN2", target_bir_lowering=False, debug=False, num_devices=NCH)

    x_d = nc.dram_tensor("x", [NBASIS, 128, 2, 2 * NXS], F16, kind="ExternalInput")
    c_d = nc.dram_tensor("csm", [128, 2, 2 * NXS], F16, kind="ExternalInput")
    v_d = nc.dram_tensor("vt", [NBASIS, NF], F32, kind="ExternalInput")
    m_d = nc.dram_tensor("mask", [NF, NX], U8, kind="ExternalInput")
    o_d = nc.dram_tensor("out", [NF, NX], F32, kind="ExternalOutput")

    fa_d = nc.inline_tensor(f_a, "fmat_a")
    fb_d = nc.inline_tensor(f_b, "fmat_b")

    # DRAM scratch for the b->partition corner turn, fp16 row-major (b, n)
    s_d = nc.dram_tensor("yscratch", [NBASIS, NX], F16)

    NBLK = 4096  # mask/out tile width
    NGATH = 8192  # yr block-read width
    NSUB = 1024  # psum projection tile width

    with tile.TileContext(nc) as tc:
        with (
            tc.tile_pool(name="const", bufs=1) as cpool,
            tc.tile_pool(name="work", bufs=1) as wpool,
            tc.tile_pool(name="psum", bufs=1, space="PSUM") as psum,
        ):
            # ---- constants ----
            fa = cpool.tile([128, 2, 512], F32R, name="fa")
            fb = cpool.tile([128, 2, 512], F32R, name="fb")
            nc.sync.dma_start(fa[:], fa_d.ap().bitcast(F32R))
            nc.sync.dma_start(fb[:], fb_d.ap().bitcast(F32R))

            csm = cpool.tile([128, 2, 2 * NXS], F16, name="csm")
            nc.sync.dma_start(csm[:], c_d.ap())
            cr = csm[:, :, 0::2]
            ci = csm[:, :, 1::2]

            vt32 = cpool.tile([NBASIS, NF], F32, name="vt32")
            nc.sync.dma_start(vt32[:], v_d.ap())
            vt16 = cpool.tile([NBASIS, NF], F16, name="vt16")
            nc.vector.tensor_copy(vt16[:], vt32[:])

            # ---- phase 1: per-image csm-multiply + 2D FFT -> DRAM scratch ----
            for b in range(NBASIS):
                xb = wpool.tile([128, 2, 2 * NXS], F16, name=f"xb{b}", tag="xb", bufs=3)
                nc.scalar.dma_start(xb[:], x_d.ap()[b])
                xr = xb[:, :, 0::2]
                xi = xb[:, :, 1::2]

                ta = wpool.tile([128, 2, NXS], F32, name=f"ta{b}", tag="ta", bufs=2)
                tb = wpool.tile([128, 2, NXS], F32, name=f"tb{b}", tag="tb", bufs=2)
                nc.vector.tensor_tensor(ta[:], xr, cr, op=MULT)
                nc.vector.tensor_tensor(tb[:], xi, ci, op=MULT)
                zr = wpool.tile([128, 2, NXS], F32R, name=f"zr{b}", tag="zr", bufs=3)
                nc.vector.tensor_tensor(zr[:], ta[:], tb[:], op=SUB)
                tc_ = wpool.tile([128, 2, NXS], F32, name=f"tc{b}", tag="tc", bufs=2)
                td = wpool.tile([128, 2, NXS], F32, name=f"td{b}", tag="td", bufs=2)
                nc.vector.tensor_tensor(tc_[:], xr, ci, op=MULT)
                nc.vector.tensor_tensor(td[:], xi, cr, op=MULT)
                zi = wpool.tile([128, 2, NXS], F32R, name=f"zi{b}", tag="zi", bufs=3)
                nc.vector.tensor_tensor(zi[:], tc_[:], td[:], op=ADD)

                # pass L: WT[j, k] = sum_i Z[i, j] T[i, k]   (W = T @ Z)
                # wt[p, qj, 0:256] = Re WT[qj*128+p, :], [256:512] = Im WT
                wt = wpool.tile([128, 2, 512], F32R, name=f"wt{b}", tag="wt", bufs=2)
                for jt in range(2):
                    js = slice(jt * 128, (jt + 1) * 128)
                    pl = psum.tile([128, 512], F32, name=f"pl{b}_{jt}", tag="pfft", bufs=2)
                    nc.tensor.matmul(pl[:], zr[:, 0, js], fa[:, 0, :], start=True, stop=False)
                    nc.tensor.matmul(pl[:], zr[:, 1, js], fa[:, 1, :], start=False, stop=False)
                    nc.tensor.matmul(pl[:], zi[:, 0, js], fb[:, 0, :], start=False, stop=False)
                    nc.tensor.matmul(pl[:], zi[:, 1, js], fb[:, 1, :], start=False, stop=True)
                    nc.scalar.copy(wt[:, jt, :], pl[:])

                # pass R: Y[k, n] = sum_j WT[j, k] T[j, n]   (Y = W @ T)
                yb = wpool.tile([128, 2, 512], F16, name=f"yb{b}", tag="yb", bufs=2)
                for kt in range(2):
                    ksr = slice(kt * 128, (kt + 1) * 128)
                    ksi = slice(256 + kt * 128, 256 + (kt + 1) * 128)
                    pr = psum.tile([128, 512], F32, name=f"pr{b}_{kt}", tag="pfft", bufs=2)
                    nc.tensor.matmul(pr[:], wt[:, 0, ksr], fa[:, 0, :], start=True, stop=False)
                    nc.tensor.matmul(pr[:], wt[:, 1, ksr], fa[:, 1, :], start=False, stop=False)
                    nc.tensor.matmul(pr[:], wt[:, 0, ksi], fb[:, 0, :], start=False, stop=False)
                    nc.tensor.matmul(pr[:], wt[:, 1, ksi], fb[:, 1, :], start=False, stop=True)
                    # interleave re/im while evicting: y[.., col*2+ri] = pr[.., ri*256+col]
                    nc.scalar.copy(
                        yb[:, kt, :].rearrange("p (c r) -> p c r", r=2),
                        pr[:].rearrange("p (r c) -> p c r", r=2),
                    )
                # scatter to scratch: scratch[b, (kt*128+p)*512 + c] = yb[p, kt, c]
                nc.scalar.dma_start(
                    s_d.ap()[b].rearrange("(kt p c) -> p kt c", p=128, kt=2),
                    yb[:],
                )

            # ---- phase 2: block read + projection + mask + store ----
            for g in range(NX // NGATH):  # 16 groups
                yg = wpool.tile([NBASIS, NGATH], F16, name=f"yg{g}", tag="yg", bufs=2)
                nc.scalar.dma_start(
                    yg[:], s_d.ap()[:, g * NGATH : (g + 1) * NGATH]
                )
                for j in range(NGATH // NBLK):  # 2 blocks per group
                    blk = g * 2 + j
                    n0 = blk * NBLK
                    m0 = wpool.tile([128, NBLK], U8, name=f"m0_{blk}", tag="m0", bufs=2)
                    m1 = wpool.tile([128, NBLK], U8, name=f"m1_{blk}", tag="m1", bufs=2)
                    nc.sync.dma_start(m0[:], m_d.ap()[0:128, n0 : n0 + NBLK])
                    nc.scalar.dma_start(m1[0:72, :], m_d.ap()[128:200, n0 : n0 + NBLK])
                    ob0 = wpool.tile([128, NBLK], F32, name=f"ob0_{blk}", tag="ob0", bufs=2)
                    ob1 = wpool.tile([128, NBLK], F32, name=f"ob1_{blk}", tag="ob1", bufs=2)
                    for ft, nf, ob, msk in ((0, 128, ob0, m0), (1, 72, ob1, m1)):
                        fsl = slice(ft * 128, ft * 128 + nf)
                        for sub in range(NBLK // NSUB):
                            off = j * NBLK + sub * NSUB
                            ssl = slice(sub * NSUB, (sub + 1) * NSUB)
                            pp = psum.tile(
                                [128, NSUB],
                                F32,
                                name=f"pp{blk}_{sub}_{ft}",
                                tag="pp",
                                bufs=3,
                            )
                            nc.tensor.matmul(
                                pp[0:nf, 0:512],
                                vt16[:, fsl],
                                yg[0:NBASIS, off : off + 512],
                                start=True,
                                stop=True,
                            )
                            nc.tensor.matmul(
                                pp[0:nf, 512:1024],
                                vt16[:, fsl],
                                yg[0:NBASIS, off + 512 : off + 1024],
                                start=True,
                                stop=True,
                            )
                            if ft == 0:
                                nc.vector.tensor_tensor(
                                    ob[0:nf, ssl], pp[0:nf, :], msk[0:nf, ssl], op=MULT
                                )
                            else:
                                t1 = wpool.tile(
                                    [128, NSUB], F32, name=f"t1_{blk}_{sub}", tag="t1", bufs=2
                                )
                                nc.scalar.copy(t1[0:nf, :], pp[0:nf, :])
                                nc.gpsimd.tensor_tensor(
                                    ob[0:nf, ssl], t1[0:nf, :], msk[0:nf, ssl], op=MULT
                                )
                    nc.sync.dma_start(o_d.ap()[0:128, n0 : n0 + NBLK], ob0[:])
                    nc.scalar.dma_start(o_d.ap()[128:200, n0 : n0 + NBLK], ob1[0:72, :])

    nc.compile()
    return nc


def _get_nc():
    if "nc" not in _CACHE:
        _CACHE["nc"] = _build()
    return _CACHE["nc"]


def kernel(x, csmT, VT, maskT):
    from concourse import bass2jax

    x = np.asarray(x, dtype=np.float32)
    x = np.ascontiguousarray(
        x.reshape(NBASIS, 2, 128, 2 * NXS).transpose(0, 2, 1, 3).astype(np.float16)
    )
    csm = np.asarray(csmT, dtype=np.float32)
    csm = np.ascontiguousarray(
        csm.reshape(8, 2, 128, 2 * NXS).transpose(0, 2, 1, 3).astype(np.float16)
    )
    vt = np.ascontiguousarray(np.asarray(VT, dtype=np.float32))
    mask = np.ascontiguousarray(np.asarray(maskT)).view(np.uint8)

    nc = _get_nc()
    in_maps = [
        {"x": x, "csm": csm[c], "vt": vt, "mask": mask} for c in range(NCH)
    ]
    results = bass2jax.run_bass_via_pjrt(nc, in_maps, n_cores=NCH)
    return np.stack([results[c]["out"] for c in range(NCH)], axis=0)
